# revision 1
# baseline (speedup 1.0000x reference)
"""BiRGAT (bipartite 2-layer GATv2) Trainium2 kernel, 8-core SPMD.

Strategy: destination-tile sharding. Gene dsts padded to 160 tiles of 128
(20 tiles/core), sample dsts 32 tiles (4/core), dealt to cores by sorted
chunk count so every core runs an identical baked per-slot chunk schedule.
Edge-phase per 128-edge chunk: indirect-DMA gather of source rows, GATv2
attention (Prelu + fused dot via scalar_tensor_tensor accum_out, exp),
one-hot matmul scatter-add of [messages | ea] into PSUM. Tile evacuation
does softmax normalization (no segment_max: alpha std ~0.3; the reference's
den+1e-16 makes max-subtraction irrelevant), bias, ELU, residuals.
Source-side tables are AllGathered between phases (overlapped with edge
compute by the Tile scheduler).
"""
import sys

sys.path.insert(0, "/opt/trn_rl_repo")

import numpy as np
from contextlib import ExitStack

import concourse.bass as bass
import concourse.tile as tile
from concourse import bacc, mybir
from concourse.bass_utils import run_bass_kernel_spmd
from concourse.masks import make_identity

P = 128
NCORES = 8
NS, NG, E = 4096, 20000, 131072
DIN, H, C1, C3 = 256, 4, 64, 128
HC1, HC3 = H * C1, H * C3          # 256, 512
NGP = 20480                        # genes padded to 160 tiles
NST, NGT = NS // P, NGP // P       # 32, 160
S_PER_CORE, G_PER_CORE = NST // NCORES, NGT // NCORES   # 4, 20
SROWS, GROWS = S_PER_CORE * P, G_PER_CORE * P           # 512, 2560

F32 = mybir.dt.float32
I32 = mybir.dt.int32
AF = mybir.ActivationFunctionType
OP = mybir.AluOpType

PAD_LOC = 200.0   # dst-local sentinel for padded edges (never equals 0..127)


# ---------------------------------------------------------------- host plan

def _deal_tiles(dst, n_tiles, per_core):
    """Deal dst tiles to cores by sorted chunk count. Returns
    assign[slot, core] -> tile id, sched[slot] -> chunks, per-tile counts."""
    tcnt = np.bincount(dst // P, minlength=n_tiles)
    chunks = np.maximum((tcnt + P - 1) // P, 1)
    order = np.argsort(-chunks, kind="stable")
    assign = order.reshape(per_core, NCORES)
    sched = chunks[assign].max(axis=1)
    return assign, sched.astype(int), tcnt


def _edge_arrays(src, dst, assign, sched, src_row_map, core):
    """Per-core edge chunk arrays for one relation.
    Returns src_rows [P, NCH] i32, dstrow [P, NCH] i32, dstloc [P, NCH] f32."""
    nch = int(sched.sum())
    src_rows = np.zeros((nch, P), np.int32)
    dstrow = np.zeros((nch, P), np.int32)
    dstloc = np.full((nch, P), PAD_LOC, np.float32)
    tile_of = dst // P
    ci = 0
    for slot in range(len(sched)):
        t = assign[slot, core]
        e = np.nonzero(tile_of == t)[0]
        n = len(e)
        want = sched[slot] * P
        s = np.zeros(want, np.int32)
        dl = np.full(want, PAD_LOC, np.float32)
        dr = np.zeros(want, np.int32)
        s[:n] = src_row_map[src[e]]
        dl[:n] = (dst[e] % P).astype(np.float32)
        dr[:n] = slot * P + dst[e] % P
        src_rows[ci:ci + sched[slot]] = s.reshape(-1, P)
        dstloc[ci:ci + sched[slot]] = dl.reshape(-1, P)
        dstrow[ci:ci + sched[slot]] = dr.reshape(-1, P)
        ci += sched[slot]
    return src_rows.T.copy(), dstrow.T.copy(), dstloc.T.copy()


def _bcast(v, p=P):
    return np.broadcast_to(np.asarray(v, np.float32).reshape(1, -1),
                           (p, len(np.asarray(v).reshape(-1)))).copy()


def _plan(inputs):
    sg_src = np.asarray(inputs["sg_src"]); sg_dst = np.asarray(inputs["sg_dst"])
    gs_src = np.asarray(inputs["gs_src"]); gs_dst = np.asarray(inputs["gs_dst"])

    g_assign, g_sched, _ = _deal_tiles(sg_dst, NGT, G_PER_CORE)
    s_assign, s_sched, _ = _deal_tiles(gs_dst, NST, S_PER_CORE)

    # tile -> (owner core, slot)
    g_owner = np.zeros(NGT, np.int32); g_slot = np.zeros(NGT, np.int32)
    for slot in range(G_PER_CORE):
        for c in range(NCORES):
            g_owner[g_assign[slot, c]] = c
            g_slot[g_assign[slot, c]] = slot
    s_owner = np.zeros(NST, np.int32); s_slot = np.zeros(NST, np.int32)
    for slot in range(S_PER_CORE):
        for c in range(NCORES):
            s_owner[s_assign[slot, c]] = c
            s_slot[s_assign[slot, c]] = slot

    sid = np.arange(NS)
    srow_tbl = s_owner[sid // P] * SROWS + s_slot[sid // P] * P + sid % P
    gid = np.arange(NG)
    grow_tbl = g_owner[gid // P] * GROWS + g_slot[gid // P] * P + gid % P

    plan = {
        "g_assign": g_assign, "g_sched": g_sched,
        "s_assign": s_assign, "s_sched": s_sched,
        "srow_tbl": srow_tbl, "grow_tbl": grow_tbl,
    }

    x_sample = np.asarray(inputs["x_sample"], np.float32)
    x_gene = np.asarray(inputs["x_gene"], np.float32)

    in_maps = []
    for c in range(NCORES):
        # node rows owned by this core, in slot order
        s_tiles = s_assign[:, c]
        xs_own = x_sample.reshape(NST, P, DIN)[s_tiles].reshape(SROWS, DIN)
        g_tiles = g_assign[:, c]
        xg_own = np.zeros((GROWS, DIN), np.float32)
        for i, t in enumerate(g_tiles):
            lo = t * P
            if lo < NG:
                n = min(P, NG - lo)
                xg_own[i * P:i * P + n] = x_gene[lo:lo + n]

        sgS, sgR, sgL = _edge_arrays(sg_src, sg_dst, g_assign, g_sched,
                                     srow_tbl, c)
        gsS, gsR, gsL = _edge_arrays(gs_src, gs_dst, s_assign, s_sched,
                                     grow_tbl, c)
        gsS3 = _edge_arrays(gs_src, gs_dst, s_assign, s_sched,
                            grow_tbl, c)[0]  # same rows; tbl3 shares layout

        m = {
            "xs_own": np.ascontiguousarray(xs_own),
            "xg_own": np.ascontiguousarray(xg_own),
            "Wl1_sg": np.asarray(inputs["Wl1_sg"], np.float32),
            "Wr1_sg": np.asarray(inputs["Wr1_sg"], np.float32),
            "Wl1_gs": np.asarray(inputs["Wl1_gs"], np.float32),
            "Wr1_gs": np.asarray(inputs["Wr1_gs"], np.float32),
            "Wl3": np.asarray(inputs["Wl3_gs"], np.float32),
            "Wr3": np.asarray(inputs["Wr3_gs"], np.float32),
            "sl1_W": np.asarray(inputs["sl1_W"], np.float32),
            "sl3_W": np.asarray(inputs["sl3_W"], np.float32),
            "att1_sg_b": _bcast(np.asarray(inputs["att1_sg"]).reshape(-1)),
            "att1_gs_b": _bcast(np.asarray(inputs["att1_gs"]).reshape(-1)),
            "att3_b": _bcast(np.asarray(inputs["att3_gs"]).reshape(-1)),
            "bl1_sg_b": _bcast(inputs["bl1_sg"]),
            "br1_sg_b": _bcast(inputs["br1_sg"]),
            "bl1_gs_b": _bcast(inputs["bl1_gs"]),
            "br1_gs_b": _bcast(inputs["br1_gs"]),
            "bias1_sg_b": _bcast(inputs["bias1_sg"]),
            "bias1_gs_b": _bcast(inputs["bias1_gs"]),
            "bl3_b": _bcast(inputs["bl3_gs"]),
            "br3_b": _bcast(inputs["br3_gs"]),
            "bias3_b": _bcast(inputs["bias3_gs"]),
            "sl1_b_b": _bcast(inputs["sl1_b"]),
            "sl3_b_b": _bcast(inputs["sl3_b"]),
            "sg_srcr": sgS, "sg_dstl": sgL,
            "gs_srcr": gsS, "gs_dstl": gsL,
            "gs_srcr3": gsS3,
        }
        in_maps.append(m)
    return plan, in_maps


# ------------------------------------------------------------- device build

def _load_w(nc, pool, w_dram, kdim, n, tag):
    """Load [kdim, n] weight into SBUF as [128, kdim//128, n] rhs tiles."""
    kc = kdim // P
    t = pool.tile([P, kc, n], F32, tag=tag)
    nc.sync.dma_start(t[:], w_dram[:].rearrange("(c p) n -> p c n", p=P))
    return t


def _transpose2(nc, sb, psp, ident, x_ap, kc):
    """PE-transpose x [128, kc*128] -> list of kc SBUF tiles [128,128]."""
    outs = []
    for k in range(kc):
        pt = psp.tile([P, P], F32, space="PSUM", tag="transp")
        nc.tensor.transpose(out=pt[:], in_=x_ap[:, k * P:(k + 1) * P],
                            identity=ident[:])
        st = sb.tile([P, P], F32, tag="transs")
        nc.scalar.copy(st[:], pt[:])
        outs.append(st)
    return outs


F32R = mybir.dt.float32r


def _r(ap):
    """Matmul operand passthrough (fp32r rejected by walrus: producers
    must pre-round; revisit if PE becomes the bottleneck)."""
    return ap


def _ap3(base_ap, h, c, mid, inner):
    """[128, h, c] view over base_ap's tensor with given free strides."""
    return bass.AP(base_ap.tensor, base_ap.offset,
                   [[base_ap.ap[0][0], P], [mid, h], [inner, c]])


def _mm_kc(nc, psum_ap, xT, w_sb, n):
    kc = len(xT)
    for k in range(kc):
        nc.tensor.matmul(psum_ap, lhsT=_r(xT[k][:]), rhs=_r(w_sb[:, k, :n]),
                         start=(k == 0), stop=(k == kc - 1))


def _elu(nc, sb, out_ap, y_ap, w):
    """out = elu(y) = (relu(y) - 1) + exp(min(y, 0)); [128, w] tiles."""
    m = sb.tile([P, w], F32, tag="elu_m")
    nc.vector.tensor_scalar(out=m[:], in0=y_ap, scalar1=0.0, scalar2=None,
                            op0=OP.min)
    e = sb.tile([P, w], F32, tag="elu_e")
    nc.scalar.activation(e[:], m[:], AF.Exp)
    r = sb.tile([P, w], F32, tag="elu_r")
    nc.scalar.activation(r[:], y_ap, AF.Relu)
    nc.vector.scalar_tensor_tensor(out=out_ap, in0=r[:], scalar=-1.0,
                                   in1=e[:], op0=OP.add, op1=OP.add)


def _build(g_sched, s_sched):
    nsg = int(g_sched.sum())
    ngs = int(s_sched.sum())
    nc = bacc.Bacc("TRN2", target_bir_lowering=False, debug=False,
                   num_devices=NCORES)

    ei = lambda name, shape, dt=F32: nc.dram_tensor(name, shape, dt,
                                                    kind="ExternalInput")
    xs_own = ei("xs_own", [SROWS, DIN]); xg_own = ei("xg_own", [GROWS, DIN])
    Wl1_sg = ei("Wl1_sg", [DIN, HC1]); Wr1_sg = ei("Wr1_sg", [DIN, HC1])
    Wl1_gs = ei("Wl1_gs", [DIN, HC1]); Wr1_gs = ei("Wr1_gs", [DIN, HC1])
    Wl3 = ei("Wl3", [HC1, HC3]); Wr3 = ei("Wr3", [HC1, HC3])
    sl1_W = ei("sl1_W", [DIN, C1]); sl3_W = ei("sl3_W", [HC1, C3])
    att1_sg_b = ei("att1_sg_b", [P, HC1]); att1_gs_b = ei("att1_gs_b", [P, HC1])
    att3_b = ei("att3_b", [P, HC3])
    bl1_sg_b = ei("bl1_sg_b", [P, HC1]); br1_sg_b = ei("br1_sg_b", [P, HC1])
    bl1_gs_b = ei("bl1_gs_b", [P, HC1]); br1_gs_b = ei("br1_gs_b", [P, HC1])
    bias1_sg_b = ei("bias1_sg_b", [P, HC1]); bias1_gs_b = ei("bias1_gs_b", [P, HC1])
    bl3_b = ei("bl3_b", [P, HC3]); br3_b = ei("br3_b", [P, HC3])
    bias3_b = ei("bias3_b", [P, C3])
    sl1_b_b = ei("sl1_b_b", [P, C1]); sl3_b_b = ei("sl3_b_b", [P, C3])
    sg_srcr = ei("sg_srcr", [P, nsg], I32)
    sg_dstl = ei("sg_dstl", [P, nsg]); gs_srcr = ei("gs_srcr", [P, ngs], I32)
    gs_dstl = ei("gs_dstl", [P, ngs])
    gs_srcr3 = ei("gs_srcr3", [P, ngs], I32)

    out_own = nc.dram_tensor("out_own", [SROWS, C3], F32, kind="ExternalOutput")

    # DRAM scratch
    agin_s = nc.dram_tensor("agin_s", [SROWS, HC1], F32R)
    agin_g = nc.dram_tensor("agin_g", [GROWS, HC1], F32R)
    agin_3 = nc.dram_tensor("agin_3", [GROWS, HC3], F32R)
    tbl_s = nc.dram_tensor("tbl_s", [NS, HC1], F32R, addr_space="Shared")
    tbl_g = nc.dram_tensor("tbl_g", [NGP, HC1], F32R, addr_space="Shared")
    tbl_3 = nc.dram_tensor("tbl_3", [NGP, HC3], F32R, addr_space="Shared")
    xr1_sg = nc.dram_tensor("xr1_sg", [GROWS, HC1], F32)
    xr1_gs = nc.dram_tensor("xr1_gs", [SROWS, HC1], F32)
    xr3 = nc.dram_tensor("xr3", [SROWS, HC3], F32)

    RG = [list(range(NCORES))]

    with tile.TileContext(nc) as tc, ExitStack() as ctx:
        res = ctx.enter_context(tc.tile_pool(name="res", bufs=1))
        wp = ctx.enter_context(tc.tile_pool(name="wp", bufs=1))
        sb = ctx.enter_context(tc.tile_pool(name="sb", bufs=6))
        ev = ctx.enter_context(tc.tile_pool(name="ev", bufs=2))
        psp = ctx.enter_context(tc.tile_pool(name="psp", bufs=2, space="PSUM"))
        ps1 = ctx.enter_context(tc.tile_pool(name="ps1", bufs=1, space="PSUM"))
        pse = ctx.enter_context(tc.tile_pool(name="pse", bufs=1, space="PSUM"))
        psx = ctx.enter_context(tc.tile_pool(name="psx", bufs=4, space="PSUM"))

        ident = res.tile([P, P], F32)
        make_identity(nc, ident[:])
        ident_r = res.tile([P, P], F32R)
        nc.scalar.copy(ident_r[:], ident[:])
        iota = res.tile([P, P], F32)
        nc.gpsimd.iota(iota[:], pattern=[[1, P]], base=0, channel_multiplier=0,
                       allow_small_or_imprecise_dtypes=True)

        def rload(name, dram, shape, dt=F32):
            t = res.tile(shape, dt, tag=name)
            nc.sync.dma_start(t[:], dram[:])
            return t

        att1_sg_t = rload("a1s", att1_sg_b, [P, HC1])
        att1_gs_t = rload("a1g", att1_gs_b, [P, HC1])
        att3_t = rload("a3", att3_b, [P, HC3])
        bias1_sg_t = rload("b1s", bias1_sg_b, [P, HC1])
        bias1_gs_t = rload("b1g", bias1_gs_b, [P, HC1])
        bias3_t = rload("b3", bias3_b, [P, C3])
        sg_srcr_t = rload("sgs", sg_srcr, [P, nsg], I32)
        sg_dstl_t = rload("sgl", sg_dstl, [P, nsg])
        gs_srcr_t = rload("gss", gs_srcr, [P, ngs], I32)
        gs_dstl_t = rload("gsl", gs_dstl, [P, ngs])
        gs_srcr3_t = rload("gs3", gs_srcr3, [P, ngs], I32)

        sl1_sb = res.tile([P, S_PER_CORE * C1], F32)   # sl1 rows per slot
        sl3_sb = res.tile([P, S_PER_CORE * C3], F32)   # sl3 rows per slot

        # weights (rhs layout [128, kc, n])
        Wl1_sg_t = _load_w(nc, wp, Wl1_sg, DIN, HC1, "Wl1_sg")
        Wr1_sg_t = _load_w(nc, wp, Wr1_sg, DIN, HC1, "Wr1_sg")
        Wl1_gs_t = _load_w(nc, wp, Wl1_gs, DIN, HC1, "Wl1_gs")
        Wr1_gs_t = _load_w(nc, wp, Wr1_gs, DIN, HC1, "Wr1_gs")
        Wl3_t = _load_w(nc, wp, Wl3, HC1, HC3, "Wl3")
        Wr3_t = _load_w(nc, wp, Wr3, HC1, HC3, "Wr3")
        sl1_W_t = _load_w(nc, wp, sl1_W, DIN, C1, "sl1_W")
        sl3_W_t = _load_w(nc, wp, sl3_W, HC1, C3, "sl3_W")
        bl1_sg_t = rload("bl1s", bl1_sg_b, [P, HC1])
        br1_sg_t = rload("br1s", br1_sg_b, [P, HC1])
        bl1_gs_t = rload("bl1g", bl1_gs_b, [P, HC1])
        br1_gs_t = rload("br1g", br1_gs_b, [P, HC1])
        bl3_t = rload("bl3", bl3_b, [P, HC3])
        br3_t = rload("br3", br3_b, [P, HC3])
        sl1_b_t = rload("sl1b", sl1_b_b, [P, C1])
        sl3_b_t = rload("sl3b", sl3_b_b, [P, C3])

        def dense_out(xT, w_sb, n, bias_t, dst_ap=None, sbuf_dst=None,
                      rdt=F32):
            pt = ps1.tile([P, n], F32, space="PSUM", tag="aux")
            _mm_kc(nc, pt[:], xT, w_sb, n)
            o = sbuf_dst if sbuf_dst is not None else sb.tile([P, n], rdt,
                                                             tag="dout")
            nc.vector.tensor_tensor(out=o[:] if sbuf_dst is None else sbuf_dst,
                                    in0=pt[:, :n], in1=bias_t[:, :n],
                                    op=OP.add)
            if dst_ap is not None:
                nc.sync.dma_start(dst_ap, o[:])
            return o

        # ---- phase A: sample node tables, AG1a
        for i in range(S_PER_CORE):
            xs = sb.tile([P, DIN], F32, tag="xnode")
            nc.sync.dma_start(xs[:], xs_own[i * P:(i + 1) * P, :])
            xT = _transpose2(nc, sb, psp, ident, xs[:], DIN // P)
            dense_out(xT, Wl1_sg_t, HC1, bl1_sg_t,
                      dst_ap=agin_s[i * P:(i + 1) * P, :], rdt=F32R)
            dense_out(xT, Wr1_gs_t, HC1, br1_gs_t,
                      dst_ap=xr1_gs[i * P:(i + 1) * P, :])
            dense_out(xT, sl1_W_t, C1, sl1_b_t,
                      sbuf_dst=sl1_sb[:, i * C1:(i + 1) * C1])
        nc.gpsimd.collective_compute("AllGather", OP.bypass, replica_groups=RG,
                                     ins=[agin_s[:]], outs=[tbl_s[:]])

        # ---- phase A: gene node tables, AG1b
        for j in range(G_PER_CORE):
            xg = sb.tile([P, DIN], F32, tag="xnode")
            nc.sync.dma_start(xg[:], xg_own[j * P:(j + 1) * P, :])
            xT = _transpose2(nc, sb, psp, ident, xg[:], DIN // P)
            dense_out(xT, Wl1_gs_t, HC1, bl1_gs_t,
                      dst_ap=agin_g[j * P:(j + 1) * P, :], rdt=F32R)
            dense_out(xT, Wr1_sg_t, HC1, br1_sg_t,
                      dst_ap=xr1_sg[j * P:(j + 1) * P, :])
        nc.gpsimd.collective_compute("AllGather", OP.bypass, replica_groups=RG,
                                     ins=[agin_g[:]], outs=[tbl_g[:]])

        # ---- edge chunk body
        def edge_chunk(ci, srcr_t, dstl_t, tbl, xr_slot, att_t,
                       psum_m, psum_d, first, last, w):
            xl = sb.tile([P, w], F32R, tag=f"xl{w}")
            nc.gpsimd.indirect_dma_start(
                out=xl[:], out_offset=None, in_=tbl[:],
                in_offset=bass.IndirectOffsetOnAxis(ap=srcr_t[:, ci:ci + 1],
                                                    axis=0))
            # one-hot of dst-local index (also kills padded edges)
            onehot = sb.tile([P, P], F32R, tag="onehot")
            nc.vector.tensor_scalar(out=onehot[:], in0=iota[:],
                                    scalar1=dstl_t[:, ci:ci + 1], scalar2=None,
                                    op0=OP.is_equal)
            # xr[e,:] = xr_slot[dstloc_e,:] via ohT.T @ xr_slot on PE
            ohTp = psp.tile([P, P], F32R, space="PSUM", tag="transp")
            nc.tensor.transpose(out=ohTp[:], in_=onehot[:], identity=ident_r[:])
            ohT = sb.tile([P, P], F32R, tag="ohT")
            nc.scalar.copy(ohT[:], ohTp[:])
            xrg = psx.tile([P, HC3], F32, space="PSUM", tag="xrg")
            nc.tensor.matmul(xrg[:, :w], lhsT=ident_r[:], rhs=xl[:],
                             start=True, stop=False)
            nc.tensor.matmul(xrg[:, :w], lhsT=ohT[:],
                             rhs=xr_slot[:, :w], start=False, stop=True)
            g = sb.tile([P, w], F32, tag=f"g{w}")
            nc.scalar.activation(g[:], xrg[:, :w], AF.Prelu, alpha=0.2)
            ch = w // H
            alpha4 = sb.tile([P, H], F32, tag="alpha4")
            junk = sb.tile([P, w // H], F32, tag="junk")
            for h in range(H):
                sl = slice(h * ch, (h + 1) * ch)
                nc.vector.scalar_tensor_tensor(
                    out=junk[:, :ch], in0=g[:, sl], scalar=1.0,
                    in1=att_t[:, sl], op0=OP.mult, op1=OP.mult,
                    accum_out=alpha4[:, h:h + 1])
            msgs = sb.tile([P, w + H], F32R, tag=f"msgs{w}")
            nc.scalar.activation(msgs[:, w:w + H], alpha4[:], AF.Exp)
            # msgs[:, :w] = xl * ea (per-head broadcast) in one 3D TT
            nc.vector.tensor_tensor(
                out=_ap3(msgs[:], H, ch, ch, 1),
                in0=_ap3(xl[:].bitcast(F32), H, ch, ch, 1),
                in1=_ap3(msgs[:, w:w + H], H, ch, 1, 0),
                op=OP.mult)
            if w == HC1:
                nc.tensor.matmul(psum_m[:], lhsT=onehot[:],
                                 rhs=msgs[:, :w + H], start=first,
                                 stop=last)
            else:
                nc.tensor.matmul(psum_m[:], lhsT=onehot[:],
                                 rhs=msgs[:, :w], start=first, stop=last)
                nc.tensor.matmul(psum_d[:], lhsT=onehot[:],
                                 rhs=msgs[:, w:w + H], start=first,
                                 stop=last)

        def norm_heads(psum_ap, den_ap, w, tag):
            """y[:, h*ch:(h+1)*ch] = psum_h / (den_h + 1e-16)."""
            ch = w // H
            den = sb.tile([P, H], F32, tag="den")
            nc.vector.tensor_scalar(out=den[:], in0=den_ap, scalar1=1e-16,
                                    scalar2=None, op0=OP.add)
            rden = sb.tile([P, H], F32, tag="rden")
            nc.vector.reciprocal(rden[:], den[:])
            y = ev.tile([P, w], F32, tag=tag)
            nc.vector.tensor_tensor(
                out=_ap3(y[:], H, ch, ch, 1),
                in0=_ap3(psum_ap, H, ch, ch, 1),
                in1=_ap3(rden[:], H, ch, 1, 0),
                op=OP.mult)
            return y, rden

        # ---- phase B: sg edges -> x1_gene -> xl3 rows, AG2
        ci = 0
        for slot in range(G_PER_CORE):
            xr_slot0 = sb.tile([P, HC1], F32, tag="xrslot1a")
            nc.sync.dma_start(xr_slot0[:], xr1_sg[slot * P:(slot + 1) * P, :])
            xr_slot = sb.tile([P, HC1], F32R, tag="xrslot1")
            nc.scalar.copy(xr_slot[:], xr_slot0[:])
            pm = pse.tile([P, HC1 + H], F32, space="PSUM", tag="pm")
            for k in range(int(g_sched[slot])):
                edge_chunk(ci, sg_srcr_t, sg_dstl_t, tbl_s,
                           xr_slot, att1_sg_t, pm, None,
                           k == 0, k == int(g_sched[slot]) - 1, HC1)
                ci += 1
            y, _ = norm_heads(pm[:], pm[:, HC1:HC1 + H], HC1, "y1g")
            y2 = ev.tile([P, HC1], F32, tag="y2g")
            nc.vector.tensor_tensor(out=y2[:], in0=y[:], in1=bias1_sg_t[:],
                                    op=OP.add)
            x1 = ev.tile([P, HC1], F32, tag="x1g")
            _elu(nc, ev, x1[:], y2[:], HC1)
            xT = _transpose2(nc, sb, psp, ident, x1[:], HC1 // P)
            dense_out(xT, Wl3_t, HC3, bl3_t,
                      dst_ap=agin_3[slot * P:(slot + 1) * P, :], rdt=F32R)
        nc.gpsimd.collective_compute("AllGather", OP.bypass, replica_groups=RG,
                                     ins=[agin_3[:]], outs=[tbl_3[:]])

        # ---- phase C: gs edges -> x1_sample -> xr3/sl3 rows
        ci = 0
        for slot in range(S_PER_CORE):
            xr_slot0 = sb.tile([P, HC1], F32, tag="xrslot1a")
            nc.sync.dma_start(xr_slot0[:], xr1_gs[slot * P:(slot + 1) * P, :])
            xr_slot = sb.tile([P, HC1], F32R, tag="xrslot1")
            nc.scalar.copy(xr_slot[:], xr_slot0[:])
            pm = pse.tile([P, HC1 + H], F32, space="PSUM", tag="pm")
            for k in range(int(s_sched[slot])):
                edge_chunk(ci, gs_srcr_t, gs_dstl_t, tbl_g,
                           xr_slot, att1_gs_t, pm, None,
                           k == 0, k == int(s_sched[slot]) - 1, HC1)
                ci += 1
            y, _ = norm_heads(pm[:], pm[:, HC1:HC1 + H], HC1, "y1s")
            y2 = ev.tile([P, HC1], F32, tag="y2s")
            nc.vector.tensor_tensor(out=y2[:], in0=y[:], in1=bias1_gs_t[:],
                                    op=OP.add)
            y3 = ev.tile([P, HC1], F32, tag="y3s")
            sl1_ap = bass.AP(sl1_sb.tensor,
                             sl1_sb[:, slot * C1:(slot + 1) * C1].offset,
                             [[sl1_sb[:].ap[0][0], P], [0, H], [1, C1]])
            y2v = bass.AP(y2.tensor, y2[:].offset,
                          [[y2[:].ap[0][0], P], [C1, H], [1, C1]])
            y3v = bass.AP(y3.tensor, y3[:].offset,
                          [[y3[:].ap[0][0], P], [C1, H], [1, C1]])
            nc.vector.tensor_tensor(out=y3v, in0=y2v, in1=sl1_ap, op=OP.add)
            x1 = ev.tile([P, HC1], F32, tag="x1s")
            _elu(nc, ev, x1[:], y3[:], HC1)
            xT = _transpose2(nc, sb, psp, ident, x1[:], HC1 // P)
            dense_out(xT, Wr3_t, HC3, br3_t,
                      dst_ap=xr3[slot * P:(slot + 1) * P, :])
            dense_out(xT, sl3_W_t, C3, sl3_b_t,
                      sbuf_dst=sl3_sb[:, slot * C3:(slot + 1) * C3])

        # ---- phase D: gs edges layer 3 -> output
        ci = 0
        for slot in range(S_PER_CORE):
            xr_slot0 = sb.tile([P, HC3], F32, tag="xrslot3a")
            nc.sync.dma_start(xr_slot0[:], xr3[slot * P:(slot + 1) * P, :])
            xr_slot = sb.tile([P, HC3], F32R, tag="xrslot3")
            nc.scalar.copy(xr_slot[:], xr_slot0[:])
            pm = pse.tile([P, HC3], F32, space="PSUM", tag="pm")
            pd = ps1.tile([P, H], F32, space="PSUM", tag="aux")
            for k in range(int(s_sched[slot])):
                edge_chunk(ci, gs_srcr3_t, gs_dstl_t, tbl_3,
                           xr_slot, att3_t, pm, pd,
                           k == 0, k == int(s_sched[slot]) - 1, HC3)
                ci += 1
            # mean over heads of psum_h / den_h == sum_h psum_h * (0.25/den_h)
            den4 = sb.tile([P, H], F32, tag="den")
            nc.vector.tensor_scalar(out=den4[:], in0=pd[:], scalar1=4.0,
                                    scalar2=4e-16, op0=OP.mult, op1=OP.add)
            rden = sb.tile([P, H], F32, tag="rden")
            nc.vector.reciprocal(rden[:], den4[:])
            base = ev.tile([P, C3], F32, tag="based")
            nc.vector.tensor_tensor(out=base[:],
                                    in0=sl3_sb[:, slot * C3:(slot + 1) * C3],
                                    in1=bias3_t[:], op=OP.add)
            accs = [base]
            for h in range(H):
                a = ev.tile([P, C3], F32, tag=f"acc{h}")
                nc.vector.scalar_tensor_tensor(
                    out=a[:], in0=pm[:, h * C3:(h + 1) * C3],
                    scalar=rden[:, h:h + 1], in1=accs[-1][:],
                    op0=OP.mult, op1=OP.add)
                accs.append(a)
            o = ev.tile([P, C3], F32, tag="outt")
            _elu(nc, ev, o[:], accs[-1][:], C3)
            nc.sync.dma_start(out_own[slot * P:(slot + 1) * P, :], o[:])

    nc.compile()
    return nc


# ------------------------------------------------------------------ driver

_CACHE = {}


def kernel(**inputs):
    plan, in_maps = _plan(inputs)
    key = (tuple(plan["g_sched"]), tuple(plan["s_sched"]))
    if key not in _CACHE:
        _CACHE[key] = _build(plan["g_sched"], plan["s_sched"])
    nc = _CACHE[key]
    r = run_bass_kernel_spmd(nc, in_maps, core_ids=list(range(NCORES)))
    out = np.zeros((NS, C3), np.float32)
    s_assign = plan["s_assign"]
    for c in range(NCORES):
        oc = r.results[c]["out_own"]
        for slot in range(S_PER_CORE):
            t = s_assign[slot, c]
            out[t * P:(t + 1) * P] = oc[slot * P:(slot + 1) * P]
    return out

